# revision 26
# baseline (speedup 1.0000x reference)
"""Trainium2 Bass kernel for batched multi-head attention (v2, fp16).

Full module:  out = softmax((X_q Wq)(X_k Wk)^T / sqrt(dh) + keymask) (X_v Wv) * qmask
Shapes: B=4, S=2048, D=1024, H=16, dh=64.

Sharding over 8 NeuronCores: core c -> (batch b = c//2, head-group g = c%2).
Each core computes batch b, heads g*8..g*8+8 (Wq/Wk/Wv column-sharded by head).
No collectives; the host scatters inputs (converted to fp16) and gathers the
[2048, 512] fp32 output blocks into the full [4, 2048, 1024] output.

v2 design (vs v1): everything fp16 on the PE.
  - fp16 matmuls run 1 row/cycle with moving-N up to 1024: half the
    instruction count of fp32r (N<=512), and 2x-faster LDWEIGHTS (FWL).
  - Heads are processed in PAIRS: even head's K/Q live on partitions 0:64,
    odd head's on 64:128, so the two S^T matmuls of a pair land on disjoint
    PE row-groups and run concurrently (measured dStart ~4ns on HW).
  - AV is one K=128 matmul per (head, kc) accumulating O^T[65, 1024] in PSUM
    (ones-column appended to VW gives the softmax denominator for free).
  - exp for BOTH heads of a pair is a single F=2048 ACTIVATE (amortizes the
    ~477ns fixed ACT instruction overhead), reading the [128, 2, 1024] S^T
    pair tile from PSUM, writing fp16 P to SBUF.
  - PSUM budget (8 banks): s_pair 4 + o0 2 + o1 2. Tail transposes reuse the
    o slots after their DVE evacuation (same tag, WAR-ordered by Tile).

Per-(pair, q-half) softmax pipeline over 16 k-chunks:
  S^T(kc)   = kw_h^T @ qwT      x2 heads (concurrent row-group pair)
  P(kc)     = exp(S^T * 0.125 + vbias)   one F=2048 ACTIVATE, fp16 out
  O^T      += [VW|1]^T @ P(kc)  x2 heads (K=128, N=1024)
Tail: evacuate O^T (DVE, fp16), PE-transpose [65,128] blocks, normalize
out = O * (qmask/denom) on DVE, DMA out.
"""

import os
import sys
import time
import threading

for _p in ("/opt/trn_rl_repo", "/opt/pypackages"):
    if _p not in sys.path and os.path.isdir(_p):
        sys.path.append(_p)

import numpy as np
from contextlib import ExitStack

import concourse.bass as bass
import concourse.tile as tile
from concourse import bacc, mybir
from concourse.bass_utils import run_bass_kernel_spmd
from concourse.masks import make_identity

B, S, D = 4, 2048, 1024
HEADS, DH = 16, 64
NEG_BIG = 1e10
N_CORES = 8
HG = HEADS // 2          # 8 heads per core
MC = HG * DH             # 512 output cols per core
NSC = S // 128           # 16 seq chunks
NDC = D // 128           # 8 contraction chunks
NMC = MC // 128          # 4 head-dim chunks (of this core's 512 cols)
NKC = NSC                # 16 key chunks
NQH = 2                  # q halves
QH = S // NQH            # 1024

F32 = mybir.dt.float32
EXP = mybir.ActivationFunctionType.Exp

# 16-bit PE dtype: fp16 (default) caps the moving operand at N=512;
# bf16 allows N=1024 (half the matmul instruction count) at ~8x coarser
# element precision. Both run 1 row/cycle on the PE.
USE_BF16 = os.environ.get("KERN_DT", "fp16") == "bf16"
F16 = mybir.dt.bfloat16 if USE_BF16 else mybir.dt.float16
MM_N = 1024 if USE_BF16 else 512
NMM = QH // MM_N         # moving chunks per q-half matmul

if USE_BF16:
    import ml_dtypes
    NP16 = ml_dtypes.bfloat16
else:
    NP16 = np.float16

# AV-side dtype: bf16 allows the P moving operand at N=1024 (one AV matmul
# per head/kc instead of two). P and VW carry ~0.4% element error vs fp16's
# 0.05%, but the S^T scores path stays fp16 so the softmax logits are clean.
BF16 = mybir.dt.bfloat16
I32 = mybir.dt.int32
AV_N = 512
NAV = QH // AV_N

# Schraudolph fast-exp offload: for these k-chunks the softmax exp runs as
#   P = bitcast_f32(i32(round(s * SCHRAU_A + b_k)))   (DVE affine -> Pool copy)
# instead of an ACT ACTIVATE, freeing the saturated ACT engine. ~2% RMS
# relative error on the offloaded P elements (3/16 of them per head; the two
# heads use disjoint chunks so ACT always has the other head's exp to chew).
# Masked keys saturate the i32 convert to -2^31 -> bitcast -0.0 -> correct.
LOG2E = 1.4426950408889634
SCHRAU_A = LOG2E * (1 << 23)
SCHRAU_B = (127.0 - 0.043677) * (1 << 23)
SCHRAU_KCS = ({2, 7, 12}, {4, 9, 14})


def _emit(tc, t):
    nc = tc.nc
    ctx = ExitStack()

    # ---------------- persistent pools ----------------
    cpool = ctx.enter_context(tc.tile_pool(name="const", bufs=1))
    # prefetch the first X_v tiles so the transpose pipeline starts early
    # (V is projected first: attention consumes vw for every k-chunk)
    xv_dram = t["xv"].ap().rearrange("(sc p) d -> sc p d", p=128)
    pre_pool = ctx.enter_context(tc.tile_pool(name="pre", bufs=1))
    pre_x = []
    for i in range(4):
        xpre = pre_pool.tile([128, D], F16, name=f"xpre{i}", tag=f"xpre{i}")
        nc.sync.dma_start(xpre[:], xv_dram[i])
        pre_x.append(xpre)

    ident = cpool.tile([128, 128], F16)
    make_identity(nc, ident[:])
    vbias = cpool.tile([128, NKC], F32)
    nc.sync.dma_start(vbias[:], t["vbias"].ap())
    sbias = cpool.tile([128, NKC], F32)
    nc.sync.dma_start(sbias[:], t["sbias"].ap())
    qmaskT = cpool.tile([128, NSC], F32)
    nc.sync.dma_start(qmaskT[:], t["qmaskT"].ap())

    qk_pool = ctx.enter_context(tc.tile_pool(name="qk", bufs=1))
    qwT = qk_pool.tile([128, NMC, S], F16)        # [m%128, mc, s]
    kwT = qk_pool.tile([128, NMC, S], F16)
    vw = qk_pool.tile([128, NKC, HG, DH + 1], F16)  # [k%128, kc, h, dh|1]
    ones = cpool.tile([128, 1], F32)
    nc.vector.memset(ones[:], 1.0)
    nc.vector.tensor_copy(                           # denominator ones column
        vw[:, :, :, DH:DH + 1], ones[:].broadcast_to([128, NKC, HG, 1])
    )

    # ---------------- projection phase ----------------
    pctx = ExitStack()
    xt_pool = pctx.enter_context(tc.tile_pool(name="xt", bufs=1))
    x_pool = pctx.enter_context(tc.tile_pool(name="x", bufs=6))
    w_pool = pctx.enter_context(tc.tile_pool(name="w", bufs=1))
    psum_t = pctx.enter_context(tc.tile_pool(name="ps_t", bufs=2, space="PSUM"))
    psum_p = pctx.enter_context(tc.tile_pool(name="ps_p", bufs=2, space="PSUM"))

    HSC = NSC // 2  # s-chunks per half

    # stage all weights: QK chunk-contiguous for fast LDWEIGHTS, V moving-major
    w_qk = {}
    for kind in ("q", "k"):
        wt = w_pool.tile([128, NDC, NMC, 128], F16, name=f"w{kind}", tag=f"w{kind}")
        nc.sync.dma_start(
            wt[:],
            t["w" + kind].ap().rearrange("(dc p) (mc m) -> p dc mc m", p=128, m=128),
        )
        w_qk[kind] = wt
    wv_sb = w_pool.tile([128, NDC, MC], F16, tag="wv")
    nc.sync.dma_start(wv_sb[:], t["wv"].ap().rearrange("(dc p) m -> p dc m", p=128))

    def emit_qk_proj(kind, sh, xt, mcs):
        dst = qwT if kind == "q" else kwT
        w_sb = w_qk[kind]
        for mcI in mcs:
            pp = psum_p.tile([128, QH], F32, tag="pp")
            for dc in range(NDC):
                for nh in range(NMM):
                    nc.tensor.matmul(
                        pp[:, nh * MM_N:(nh + 1) * MM_N],
                        w_sb[:, dc, mcI, :],
                        xt[:, dc, nh * MM_N:(nh + 1) * MM_N],
                        start=(dc == 0),
                        stop=(dc == NDC - 1),
                    )
            nc.vector.tensor_copy(dst[:, mcI, sh * QH:(sh + 1) * QH], pp[:])

    for xname, kind in (("xv", "v"), ("xk", "k"), ("xq", "q")):
        x_dram = t[xname].ap().rearrange("(sc p) d -> sc p d", p=128)
        for sh in range(2):  # s-halves
            # transpose this half of X into xt [d%128, dc, s_local]
            xt = xt_pool.tile([128, NDC, QH], F16, tag="xt",
                              name=f"xt_{kind}_{sh}")
            for scl in range(HSC):
                sc = sh * HSC + scl
                if xname == "xv" and sh == 0 and scl < len(pre_x):
                    xt_in = pre_x[scl]
                else:
                    xt_in = x_pool.tile([128, D], F16, tag="x")
                    nc.sync.dma_start(xt_in[:], x_dram[sc])
                pt = psum_t.tile([128, NDC, 128], F16, tag="pt")
                for dc in range(NDC):
                    nc.tensor.transpose(
                        pt[:, dc, :], xt_in[:, dc * 128:(dc + 1) * 128], ident[:]
                    )
                nc.vector.tensor_copy(xt[:, :, scl * 128:(scl + 1) * 128], pt[:])

            if kind in ("q", "k"):
                emit_qk_proj(kind, sh, xt, range(NMC))
            else:
                for scl in range(HSC):
                    sc = sh * HSC + scl
                    pv = psum_p.tile([128, MC], F32, tag="pv")
                    for dc in range(NDC):
                        nc.tensor.matmul(
                            pv[:],
                            xt[:, dc, scl * 128:(scl + 1) * 128],
                            wv_sb[:, dc, :],
                            start=(dc == 0),
                            stop=(dc == NDC - 1),
                        )
                    nc.vector.tensor_copy(
                        vw[:, sc, :, 0:DH],
                        pv[:].rearrange("p (h d) -> p h d", h=HG),
                    )

    pctx.close()

    # ---------------- attention phase ----------------
    actx = ExitStack()
    p_pool = actx.enter_context(tc.tile_pool(name="p", bufs=3))
    y_pool = actx.enter_context(tc.tile_pool(name="y", bufs=2))
    ot_pool = actx.enter_context(tc.tile_pool(name="ot", bufs=4))
    rq_pool = actx.enter_context(tc.tile_pool(name="rq", bufs=2))
    out_pool = actx.enter_context(tc.tile_pool(name="out", bufs=3))
    psum_s = actx.enter_context(tc.tile_pool(name="ps_s", bufs=2, space="PSUM"))
    psum_o = actx.enter_context(tc.tile_pool(name="ps_o", bufs=2, space="PSUM"))

    # DRAM view: [qh, p, qb, h, d] for per-(head, q-half) strip stores
    out_v = t["out"].ap().rearrange(
        "(a qb p) (hh d) -> a p qb hh d", a=NQH, p=128, hh=HG
    )

    # deferred tail work (PE transposes + normalize) from the previous
    # (pair, qh) iteration; flushed early in the next iteration so the
    # transposes fill the PE gap while ACT streams the first exps.
    tails = []

    def flush_tail():
        while tails:
            tails.pop(0)()

    for hp in range(HG // 2):
        h0, h1 = 2 * hp, 2 * hp + 1
        mcI = hp                      # mc_h = h // 2 == hp for both heads
        kw0 = kwT[0:64, mcI, :]
        kw1 = kwT[64:128, mcI, :]
        qw0 = qwT[0:64, mcI, :]
        qw1 = qwT[64:128, mcI, :]
        for qh in range(NQH):
            q0 = qh * QH
            # per-head S^T tiles (2 banks each) and P tiles; the two heads'
            # exps are STAGGERED on ACT so each head's S(kc+1)/AV(kc) runs on
            # the PE under the OTHER head's exp -> ACT never idles.
            s_t = [
                psum_s.tile([128, QH], F32, tag="s", name=f"s{i}_{hp}_{qh}")
                for i in range(2)
            ]
            kwh = (kw0, kw1)
            qwh = (qw0, qw1)

            def emit_S(i, kc):
                for nh in range(NMM):
                    nc.tensor.matmul(
                        s_t[i][:, nh * MM_N:(nh + 1) * MM_N],
                        kwh[i][:, kc * 128:(kc + 1) * 128],
                        qwh[i][:, q0 + nh * MM_N:q0 + (nh + 1) * MM_N],
                        start=True, stop=True,
                    )

            def emit_exp(i, kc):
                p_t = p_pool.tile([128, QH], F16, tag="p",
                                  name=f"p{i}_{hp}_{qh}_{kc}")
                if kc in SCHRAU_KCS[i]:
                    yi = y_pool.tile([128, QH], I32, tag="y",
                                     name=f"y{i}_{hp}_{qh}_{kc}")
                    nc.vector.tensor_scalar(
                        yi[:], s_t[i][:], SCHRAU_A * 0.125,
                        sbias[:, kc:kc + 1],
                        mybir.AluOpType.mult, mybir.AluOpType.add,
                    )
                    nc.gpsimd.tensor_copy(p_t[:], yi[:].bitcast(F32))
                else:
                    nc.scalar.activation(
                        p_t[:], s_t[i][:], EXP,
                        bias=vbias[:, kc:kc + 1], scale=0.125,
                    )
                return p_t

            emit_S(0, 0)
            emit_S(1, 0)
            p0 = emit_exp(0, 0)
            # previous iteration's transposes/normalize fill the PE bubble
            # while ACT runs this iteration's first exps
            flush_tail()
            o_t = [
                psum_o.tile([DH + 1, QH], F32, tag="o", name=f"o{i}_{hp}_{qh}")
                for i in range(2)
            ]

            def emit_av(i, kc, p_t):
                first, last = kc == 0, kc == NKC - 1
                for nh in range(NAV):
                    nc.tensor.matmul(
                        o_t[i][:, nh * AV_N:(nh + 1) * AV_N],
                        vw[:, kc, 2 * hp + i, :],
                        p_t[:, nh * AV_N:(nh + 1) * AV_N],
                        start=first, stop=last,
                    )

            for kc in range(NKC):
                if kc > 0:
                    p0 = emit_exp(0, kc)
                if kc + 1 < NKC:
                    emit_S(0, kc + 1)
                emit_av(0, kc, p0)
                p1 = emit_exp(1, kc)
                if kc + 1 < NKC:
                    emit_S(1, kc + 1)
                emit_av(1, kc, p1)

            # evacuate O^T now (frees the o slots for the next iteration);
            # defer transposes + normalize into the next iteration's start.
            evac = []
            for i in range(2):
                ot = ot_pool.tile([DH + 1, QH], F16, tag="ot",
                                  name=f"ot_{2 * hp + i}_{qh}")
                nc.vector.tensor_copy(ot[:], o_t[i][:])
                evac.append(ot)

            def make_tail(hp=hp, qh=qh, evac=evac):
                def tail():
                    for hloc, ot in ((2 * hp, evac[0]), (2 * hp + 1, evac[1])):
                        # inner dim padded to DH+2 so each [*, qb, :] slice
                        # lands 4-byte aligned in PSUM (fp16 elements)
                        tr = psum_o.tile([128, 8, DH + 2], F16, tag="o",
                                         name=f"tr_{hloc}_{qh}")
                        for qb in range(8):
                            nc.tensor.transpose(
                                tr[:, qb, 0:DH + 1],
                                ot[:, qb * 128:(qb + 1) * 128],
                                ident[0:DH + 1, 0:DH + 1],
                            )
                        # normalize: out = O * qmask/denom (denom = col 64)
                        rq = rq_pool.tile([128, 8], F32, tag="rq",
                                          name=f"rq_{hloc}_{qh}")
                        nc.vector.reciprocal(rq[:], tr[:, :, DH])
                        nc.vector.tensor_mul(
                            rq[:], rq[:], qmaskT[:, qh * 8:(qh + 1) * 8]
                        )
                        ob = out_pool.tile([128, 8, DH], F32, tag="ob",
                                           name=f"ob_{hloc}_{qh}")
                        nc.vector.tensor_mul(
                            ob[:], tr[:, :, 0:DH],
                            rq[:].broadcast_to([128, 8, DH]),
                        )
                        nc.sync.dma_start(out_v[qh][:, :, hloc, :], ob[:])
                return tail

            tails.append(make_tail())

    flush_tail()
    actx.close()
    ctx.close()


_BUILD_LOCK = threading.Lock()
_CACHE = {}


def _build():
    with _BUILD_LOCK:
        if "nc" in _CACHE:
            return _CACHE["nc"]
        nc = bacc.Bacc(
            "TRN2", target_bir_lowering=False, debug=False, num_devices=N_CORES
        )
        t = {
            "xq": nc.dram_tensor("xq", [S, D], F16, kind="ExternalInput"),
            "xk": nc.dram_tensor("xk", [S, D], F16, kind="ExternalInput"),
            "xv": nc.dram_tensor("xv", [S, D], F16, kind="ExternalInput"),
            "wq": nc.dram_tensor("wq", [D, MC], F16, kind="ExternalInput"),
            "wk": nc.dram_tensor("wk", [D, MC], F16, kind="ExternalInput"),
            "wv": nc.dram_tensor("wv", [D, MC], F16, kind="ExternalInput"),
            "vbias": nc.dram_tensor("vbias", [128, NKC], F32, kind="ExternalInput"),
            "sbias": nc.dram_tensor("sbias", [128, NKC], F32, kind="ExternalInput"),
            "qmaskT": nc.dram_tensor("qmaskT", [128, NSC], F32, kind="ExternalInput"),
            "out": nc.dram_tensor("out", [S, MC], F32, kind="ExternalOutput"),
        }
        with tile.TileContext(nc) as tc:
            _emit(tc, t)
        nc.compile()
        _CACHE["nc"] = nc
        return nc


def _in_maps(q_value, k_value, v_value, v_mask, q_mask, Wq, Wk, Wv):
    maps = []
    for c in range(N_CORES):
        b, g = c // 2, c % 2
        m0 = g * MC
        vb = ((v_mask[b, :, 0].reshape(NKC, 128).T) - 1.0) * NEG_BIG
        qm = q_mask[b, :, 0].reshape(NSC, 128).T
        maps.append({
            "xq": np.ascontiguousarray(q_value[b]).astype(NP16),
            "xk": np.ascontiguousarray(k_value[b]).astype(NP16),
            "xv": np.ascontiguousarray(v_value[b]).astype(NP16),
            "wq": np.ascontiguousarray(Wq[:, m0:m0 + MC]).astype(NP16),
            "wk": np.ascontiguousarray(Wk[:, m0:m0 + MC]).astype(NP16),
            "wv": np.ascontiguousarray(Wv[:, m0:m0 + MC]).astype(NP16),
            "vbias": np.ascontiguousarray(vb).astype(np.float32),
            "sbias": np.ascontiguousarray(
                vb * 0.125 * SCHRAU_A + SCHRAU_B).astype(np.float32),
            "qmaskT": np.ascontiguousarray(qm).astype(np.float32),
        })
    return maps


def _assemble(results):
    out = np.empty((B, S, HEADS * DH), dtype=np.float32)
    for c in range(N_CORES):
        b, g = c // 2, c % 2
        out[b, :, g * MC:(g + 1) * MC] = results[c]["out"]
    return out


def kernel(q_value, k_value, v_value, v_mask, q_mask, Wq, Wk, Wv,
           profile=False, trace_cores=None):
    nc = _build()
    maps = _in_maps(np.asarray(q_value, dtype=np.float32),
                    np.asarray(k_value, dtype=np.float32),
                    np.asarray(v_value, dtype=np.float32),
                    np.asarray(v_mask, dtype=np.float32),
                    np.asarray(q_mask, dtype=np.float32),
                    np.asarray(Wq, dtype=np.float32),
                    np.asarray(Wk, dtype=np.float32),
                    np.asarray(Wv, dtype=np.float32))
    if profile:
        _install_profile_hook()
    res = run_bass_kernel_spmd(
        nc, maps, list(range(N_CORES)),
        trace=profile, trace_cores=trace_cores,
    )
    out = _assemble(res.results)
    if profile:
        return out, res
    return out


def _install_profile_hook():
    """Wire up the NTFF profile hook that this container image lacks."""
    import types
    if "antenv.axon_hooks" in sys.modules:
        return
    try:
        from trn_agent_boot.trn_boot import _ntff_profile_via_ctypes
        hook = _ntff_profile_via_ctypes("/opt/axon/libaxon_pjrt.so")
    except Exception:
        hook = None
    mod = types.ModuleType("antenv.axon_hooks")
    mod.get_axon_ntff_profile_hook = lambda: hook
    sys.modules["antenv.axon_hooks"] = mod


if __name__ == "__main__":
    t0 = time.time()
    _build()
    print(f"build+compile: {time.time() - t0:.1f}s")


# revision 27
# speedup vs baseline: 1.4520x; 1.4520x over previous
"""Trainium2 Bass kernel for batched multi-head attention (v2, fp16).

Full module:  out = softmax((X_q Wq)(X_k Wk)^T / sqrt(dh) + keymask) (X_v Wv) * qmask
Shapes: B=4, S=2048, D=1024, H=16, dh=64.

Sharding over 8 NeuronCores: core c -> (batch b = c//2, head-group g = c%2).
Each core computes batch b, heads g*8..g*8+8 (Wq/Wk/Wv column-sharded by head).
No collectives; the host scatters inputs (converted to fp16) and gathers the
[2048, 512] fp32 output blocks into the full [4, 2048, 1024] output.

v2 design (vs v1): everything fp16 on the PE.
  - fp16 matmuls run 1 row/cycle with moving-N up to 1024: half the
    instruction count of fp32r (N<=512), and 2x-faster LDWEIGHTS (FWL).
  - Heads are processed in PAIRS: even head's K/Q live on partitions 0:64,
    odd head's on 64:128, so the two S^T matmuls of a pair land on disjoint
    PE row-groups and run concurrently (measured dStart ~4ns on HW).
  - AV is one K=128 matmul per (head, kc) accumulating O^T[65, 1024] in PSUM
    (ones-column appended to VW gives the softmax denominator for free).
  - exp for BOTH heads of a pair is a single F=2048 ACTIVATE (amortizes the
    ~477ns fixed ACT instruction overhead), reading the [128, 2, 1024] S^T
    pair tile from PSUM, writing fp16 P to SBUF.
  - PSUM budget (8 banks): s_pair 4 + o0 2 + o1 2. Tail transposes reuse the
    o slots after their DVE evacuation (same tag, WAR-ordered by Tile).

Per-(pair, q-half) softmax pipeline over 16 k-chunks:
  S^T(kc)   = kw_h^T @ qwT      x2 heads (concurrent row-group pair)
  P(kc)     = exp(S^T * 0.125 + vbias)   one F=2048 ACTIVATE, fp16 out
  O^T      += [VW|1]^T @ P(kc)  x2 heads (K=128, N=1024)
Tail: evacuate O^T (DVE, fp16), PE-transpose [65,128] blocks, normalize
out = O * (qmask/denom) on DVE, DMA out.
"""

import os
import sys
import time
import threading

for _p in ("/opt/trn_rl_repo", "/opt/pypackages"):
    if _p not in sys.path and os.path.isdir(_p):
        sys.path.append(_p)

import numpy as np
from contextlib import ExitStack

import concourse.bass as bass
import concourse.tile as tile
from concourse import bacc, mybir
from concourse.bass_utils import run_bass_kernel_spmd
from concourse.masks import make_identity

B, S, D = 4, 2048, 1024
HEADS, DH = 16, 64
NEG_BIG = 1e10
N_CORES = 8
HG = HEADS // 2          # 8 heads per core
MC = HG * DH             # 512 output cols per core
NSC = S // 128           # 16 seq chunks
NDC = D // 128           # 8 contraction chunks
NMC = MC // 128          # 4 head-dim chunks (of this core's 512 cols)
NKC = NSC                # 16 key chunks
NQH = 2                  # q halves
QH = S // NQH            # 1024

F32 = mybir.dt.float32
EXP = mybir.ActivationFunctionType.Exp

# 16-bit PE dtype: fp16 (default) caps the moving operand at N=512;
# bf16 allows N=1024 (half the matmul instruction count) at ~8x coarser
# element precision. Both run 1 row/cycle on the PE.
USE_BF16 = os.environ.get("KERN_DT", "fp16") == "bf16"
F16 = mybir.dt.bfloat16 if USE_BF16 else mybir.dt.float16
MM_N = 1024 if USE_BF16 else 512
NMM = QH // MM_N         # moving chunks per q-half matmul

if USE_BF16:
    import ml_dtypes
    NP16 = ml_dtypes.bfloat16
else:
    NP16 = np.float16

# AV-side dtype: bf16 allows the P moving operand at N=1024 (one AV matmul
# per head/kc instead of two). P and VW carry ~0.4% element error vs fp16's
# 0.05%, but the S^T scores path stays fp16 so the softmax logits are clean.
BF16 = mybir.dt.bfloat16
I32 = mybir.dt.int32
AV_N = 512
NAV = QH // AV_N

# Schraudolph fast-exp offload: for these k-chunks the softmax exp runs as
#   P = bitcast_f32(i32(round(s * SCHRAU_A + b_k)))   (DVE affine -> Pool copy)
# instead of an ACT ACTIVATE, freeing the saturated ACT engine. ~2% RMS
# relative error on the offloaded P elements (3/16 of them per head; the two
# heads use disjoint chunks so ACT always has the other head's exp to chew).
# Masked keys saturate the i32 convert to -2^31 -> bitcast -0.0 -> correct.
LOG2E = 1.4426950408889634
SCHRAU_A = LOG2E * (1 << 23)
SCHRAU_B = (127.0 - 0.043677) * (1 << 23)
# off by default: DVE+Pool measured slower per element than ACT on HW, and
# the extra engine activity worsened power throttling
if os.environ.get("SCHRAU", "0") == "1":
    SCHRAU_KCS = ({2, 7, 12}, {4, 9, 14})
else:
    SCHRAU_KCS = (set(), set())


def _emit(tc, t):
    nc = tc.nc
    ctx = ExitStack()

    # ---------------- persistent pools ----------------
    cpool = ctx.enter_context(tc.tile_pool(name="const", bufs=1))
    # prefetch the first X_v tiles so the transpose pipeline starts early
    # (V is projected first: attention consumes vw for every k-chunk)
    xv_dram = t["xv"].ap().rearrange("(sc p) d -> sc p d", p=128)
    pre_pool = ctx.enter_context(tc.tile_pool(name="pre", bufs=1))
    pre_x = []
    for i in range(4):
        xpre = pre_pool.tile([128, D], F16, name=f"xpre{i}", tag=f"xpre{i}")
        nc.sync.dma_start(xpre[:], xv_dram[i])
        pre_x.append(xpre)

    ident = cpool.tile([128, 128], F16)
    make_identity(nc, ident[:])
    vbias = cpool.tile([128, NKC], F32)
    nc.sync.dma_start(vbias[:], t["vbias"].ap())
    sbias = cpool.tile([128, NKC], F32)
    nc.sync.dma_start(sbias[:], t["sbias"].ap())
    qmaskT = cpool.tile([128, NSC], F32)
    nc.sync.dma_start(qmaskT[:], t["qmaskT"].ap())

    qk_pool = ctx.enter_context(tc.tile_pool(name="qk", bufs=1))
    qwT = qk_pool.tile([128, NMC, S], F16)        # [m%128, mc, s]
    kwT = qk_pool.tile([128, NMC, S], F16)
    vw = qk_pool.tile([128, NKC, HG, DH + 1], F16)  # [k%128, kc, h, dh|1]
    ones = cpool.tile([128, 1], F32)
    nc.vector.memset(ones[:], 1.0)
    nc.vector.tensor_copy(                           # denominator ones column
        vw[:, :, :, DH:DH + 1], ones[:].broadcast_to([128, NKC, HG, 1])
    )

    # ---------------- projection phase ----------------
    pctx = ExitStack()
    xt_pool = pctx.enter_context(tc.tile_pool(name="xt", bufs=1))
    x_pool = pctx.enter_context(tc.tile_pool(name="x", bufs=6))
    w_pool = pctx.enter_context(tc.tile_pool(name="w", bufs=1))
    psum_t = pctx.enter_context(tc.tile_pool(name="ps_t", bufs=2, space="PSUM"))
    psum_p = pctx.enter_context(tc.tile_pool(name="ps_p", bufs=2, space="PSUM"))

    HSC = NSC // 2  # s-chunks per half

    # stage all weights: QK chunk-contiguous for fast LDWEIGHTS, V moving-major
    w_qk = {}
    for kind in ("q", "k"):
        wt = w_pool.tile([128, NDC, NMC, 128], F16, name=f"w{kind}", tag=f"w{kind}")
        nc.sync.dma_start(
            wt[:],
            t["w" + kind].ap().rearrange("(dc p) (mc m) -> p dc mc m", p=128, m=128),
        )
        w_qk[kind] = wt
    wv_sb = w_pool.tile([128, NDC, MC], F16, tag="wv")
    nc.sync.dma_start(wv_sb[:], t["wv"].ap().rearrange("(dc p) m -> p dc m", p=128))

    def emit_qk_proj(kind, sh, xt, mcs):
        dst = qwT if kind == "q" else kwT
        w_sb = w_qk[kind]
        for mcI in mcs:
            pp = psum_p.tile([128, QH], F32, tag="pp")
            for dc in range(NDC):
                for nh in range(NMM):
                    nc.tensor.matmul(
                        pp[:, nh * MM_N:(nh + 1) * MM_N],
                        w_sb[:, dc, mcI, :],
                        xt[:, dc, nh * MM_N:(nh + 1) * MM_N],
                        start=(dc == 0),
                        stop=(dc == NDC - 1),
                    )
            nc.vector.tensor_copy(dst[:, mcI, sh * QH:(sh + 1) * QH], pp[:])

    for xname, kind in (("xv", "v"), ("xk", "k"), ("xq", "q")):
        x_dram = t[xname].ap().rearrange("(sc p) d -> sc p d", p=128)
        for sh in range(2):  # s-halves
            # transpose this half of X into xt [d%128, dc, s_local]
            xt = xt_pool.tile([128, NDC, QH], F16, tag="xt",
                              name=f"xt_{kind}_{sh}")
            for scl in range(HSC):
                sc = sh * HSC + scl
                if xname == "xv" and sh == 0 and scl < len(pre_x):
                    xt_in = pre_x[scl]
                else:
                    xt_in = x_pool.tile([128, D], F16, tag="x")
                    nc.sync.dma_start(xt_in[:], x_dram[sc])
                pt = psum_t.tile([128, NDC, 128], F16, tag="pt")
                for dc in range(NDC):
                    nc.tensor.transpose(
                        pt[:, dc, :], xt_in[:, dc * 128:(dc + 1) * 128], ident[:]
                    )
                nc.vector.tensor_copy(xt[:, :, scl * 128:(scl + 1) * 128], pt[:])

            if kind in ("q", "k"):
                emit_qk_proj(kind, sh, xt, range(NMC))
            else:
                for scl in range(HSC):
                    sc = sh * HSC + scl
                    pv = psum_p.tile([128, MC], F32, tag="pv")
                    for dc in range(NDC):
                        nc.tensor.matmul(
                            pv[:],
                            xt[:, dc, scl * 128:(scl + 1) * 128],
                            wv_sb[:, dc, :],
                            start=(dc == 0),
                            stop=(dc == NDC - 1),
                        )
                    nc.vector.tensor_copy(
                        vw[:, sc, :, 0:DH],
                        pv[:].rearrange("p (h d) -> p h d", h=HG),
                    )

    pctx.close()

    # ---------------- attention phase ----------------
    actx = ExitStack()
    p_pool = actx.enter_context(tc.tile_pool(name="p", bufs=3))
    y_pool = actx.enter_context(tc.tile_pool(name="y", bufs=2))
    ot_pool = actx.enter_context(tc.tile_pool(name="ot", bufs=4))
    rq_pool = actx.enter_context(tc.tile_pool(name="rq", bufs=2))
    out_pool = actx.enter_context(tc.tile_pool(name="out", bufs=3))
    psum_s = actx.enter_context(tc.tile_pool(name="ps_s", bufs=2, space="PSUM"))
    psum_o = actx.enter_context(tc.tile_pool(name="ps_o", bufs=2, space="PSUM"))

    # DRAM view: [qh, p, qb, h, d] for per-(head, q-half) strip stores
    out_v = t["out"].ap().rearrange(
        "(a qb p) (hh d) -> a p qb hh d", a=NQH, p=128, hh=HG
    )

    # deferred tail work (PE transposes + normalize) from the previous
    # (pair, qh) iteration; flushed early in the next iteration so the
    # transposes fill the PE gap while ACT streams the first exps.
    tails = []

    def flush_tail():
        while tails:
            tails.pop(0)()

    for hp in range(HG // 2):
        h0, h1 = 2 * hp, 2 * hp + 1
        mcI = hp                      # mc_h = h // 2 == hp for both heads
        kw0 = kwT[0:64, mcI, :]
        kw1 = kwT[64:128, mcI, :]
        qw0 = qwT[0:64, mcI, :]
        qw1 = qwT[64:128, mcI, :]
        for qh in range(NQH):
            q0 = qh * QH
            # per-head S^T tiles (2 banks each) and P tiles; the two heads'
            # exps are STAGGERED on ACT so each head's S(kc+1)/AV(kc) runs on
            # the PE under the OTHER head's exp -> ACT never idles.
            s_t = [
                psum_s.tile([128, QH], F32, tag="s", name=f"s{i}_{hp}_{qh}")
                for i in range(2)
            ]
            kwh = (kw0, kw1)
            qwh = (qw0, qw1)

            def emit_S(i, kc):
                for nh in range(NMM):
                    nc.tensor.matmul(
                        s_t[i][:, nh * MM_N:(nh + 1) * MM_N],
                        kwh[i][:, kc * 128:(kc + 1) * 128],
                        qwh[i][:, q0 + nh * MM_N:q0 + (nh + 1) * MM_N],
                        start=True, stop=True,
                    )

            def emit_exp(i, kc):
                p_t = p_pool.tile([128, QH], F16, tag="p",
                                  name=f"p{i}_{hp}_{qh}_{kc}")
                if kc in SCHRAU_KCS[i]:
                    yi = y_pool.tile([128, QH], I32, tag="y",
                                     name=f"y{i}_{hp}_{qh}_{kc}")
                    nc.vector.tensor_scalar(
                        yi[:], s_t[i][:], SCHRAU_A * 0.125,
                        sbias[:, kc:kc + 1],
                        mybir.AluOpType.mult, mybir.AluOpType.add,
                    )
                    nc.gpsimd.tensor_copy(p_t[:], yi[:].bitcast(F32))
                else:
                    nc.scalar.activation(
                        p_t[:], s_t[i][:], EXP,
                        bias=vbias[:, kc:kc + 1], scale=0.125,
                    )
                return p_t

            emit_S(0, 0)
            emit_S(1, 0)
            p0 = emit_exp(0, 0)
            # previous iteration's transposes/normalize fill the PE bubble
            # while ACT runs this iteration's first exps
            flush_tail()
            o_t = [
                psum_o.tile([DH + 1, QH], F32, tag="o", name=f"o{i}_{hp}_{qh}")
                for i in range(2)
            ]

            def emit_av(i, kc, p_t):
                first, last = kc == 0, kc == NKC - 1
                for nh in range(NAV):
                    nc.tensor.matmul(
                        o_t[i][:, nh * AV_N:(nh + 1) * AV_N],
                        vw[:, kc, 2 * hp + i, :],
                        p_t[:, nh * AV_N:(nh + 1) * AV_N],
                        start=first, stop=last,
                    )

            for kc in range(NKC):
                if kc > 0:
                    p0 = emit_exp(0, kc)
                if kc + 1 < NKC:
                    emit_S(0, kc + 1)
                emit_av(0, kc, p0)
                p1 = emit_exp(1, kc)
                if kc + 1 < NKC:
                    emit_S(1, kc + 1)
                emit_av(1, kc, p1)

            # evacuate O^T now (frees the o slots for the next iteration);
            # defer transposes + normalize into the next iteration's start.
            evac = []
            for i in range(2):
                ot = ot_pool.tile([DH + 1, QH], F16, tag="ot",
                                  name=f"ot_{2 * hp + i}_{qh}")
                nc.vector.tensor_copy(ot[:], o_t[i][:])
                evac.append(ot)

            def make_tail(hp=hp, qh=qh, evac=evac):
                def tail():
                    for hloc, ot in ((2 * hp, evac[0]), (2 * hp + 1, evac[1])):
                        # inner dim padded to DH+2 so each [*, qb, :] slice
                        # lands 4-byte aligned in PSUM (fp16 elements)
                        tr = psum_o.tile([128, 8, DH + 2], F16, tag="o",
                                         name=f"tr_{hloc}_{qh}")
                        for qb in range(8):
                            nc.tensor.transpose(
                                tr[:, qb, 0:DH + 1],
                                ot[:, qb * 128:(qb + 1) * 128],
                                ident[0:DH + 1, 0:DH + 1],
                            )
                        # normalize: out = O * qmask/denom (denom = col 64)
                        rq = rq_pool.tile([128, 8], F32, tag="rq",
                                          name=f"rq_{hloc}_{qh}")
                        nc.vector.reciprocal(rq[:], tr[:, :, DH])
                        nc.vector.tensor_mul(
                            rq[:], rq[:], qmaskT[:, qh * 8:(qh + 1) * 8]
                        )
                        ob = out_pool.tile([128, 8, DH], F32, tag="ob",
                                           name=f"ob_{hloc}_{qh}")
                        nc.vector.tensor_mul(
                            ob[:], tr[:, :, 0:DH],
                            rq[:].broadcast_to([128, 8, DH]),
                        )
                        nc.sync.dma_start(out_v[qh][:, :, hloc, :], ob[:])
                return tail

            tails.append(make_tail())

    flush_tail()
    actx.close()
    ctx.close()


_BUILD_LOCK = threading.Lock()
_CACHE = {}


def _build():
    with _BUILD_LOCK:
        if "nc" in _CACHE:
            return _CACHE["nc"]
        nc = bacc.Bacc(
            "TRN2", target_bir_lowering=False, debug=False, num_devices=N_CORES
        )
        t = {
            "xq": nc.dram_tensor("xq", [S, D], F16, kind="ExternalInput"),
            "xk": nc.dram_tensor("xk", [S, D], F16, kind="ExternalInput"),
            "xv": nc.dram_tensor("xv", [S, D], F16, kind="ExternalInput"),
            "wq": nc.dram_tensor("wq", [D, MC], F16, kind="ExternalInput"),
            "wk": nc.dram_tensor("wk", [D, MC], F16, kind="ExternalInput"),
            "wv": nc.dram_tensor("wv", [D, MC], F16, kind="ExternalInput"),
            "vbias": nc.dram_tensor("vbias", [128, NKC], F32, kind="ExternalInput"),
            "sbias": nc.dram_tensor("sbias", [128, NKC], F32, kind="ExternalInput"),
            "qmaskT": nc.dram_tensor("qmaskT", [128, NSC], F32, kind="ExternalInput"),
            "out": nc.dram_tensor("out", [S, MC], F32, kind="ExternalOutput"),
        }
        with tile.TileContext(nc) as tc:
            _emit(tc, t)
        nc.compile()
        _CACHE["nc"] = nc
        return nc


def _in_maps(q_value, k_value, v_value, v_mask, q_mask, Wq, Wk, Wv):
    maps = []
    for c in range(N_CORES):
        b, g = c // 2, c % 2
        m0 = g * MC
        vb = ((v_mask[b, :, 0].reshape(NKC, 128).T) - 1.0) * NEG_BIG
        qm = q_mask[b, :, 0].reshape(NSC, 128).T
        maps.append({
            "xq": np.ascontiguousarray(q_value[b]).astype(NP16),
            "xk": np.ascontiguousarray(k_value[b]).astype(NP16),
            "xv": np.ascontiguousarray(v_value[b]).astype(NP16),
            "wq": np.ascontiguousarray(Wq[:, m0:m0 + MC]).astype(NP16),
            "wk": np.ascontiguousarray(Wk[:, m0:m0 + MC]).astype(NP16),
            "wv": np.ascontiguousarray(Wv[:, m0:m0 + MC]).astype(NP16),
            "vbias": np.ascontiguousarray(vb).astype(np.float32),
            "sbias": np.ascontiguousarray(
                vb * 0.125 * SCHRAU_A + SCHRAU_B).astype(np.float32),
            "qmaskT": np.ascontiguousarray(qm).astype(np.float32),
        })
    return maps


def _assemble(results):
    out = np.empty((B, S, HEADS * DH), dtype=np.float32)
    for c in range(N_CORES):
        b, g = c // 2, c % 2
        out[b, :, g * MC:(g + 1) * MC] = results[c]["out"]
    return out


def kernel(q_value, k_value, v_value, v_mask, q_mask, Wq, Wk, Wv,
           profile=False, trace_cores=None):
    nc = _build()
    maps = _in_maps(np.asarray(q_value, dtype=np.float32),
                    np.asarray(k_value, dtype=np.float32),
                    np.asarray(v_value, dtype=np.float32),
                    np.asarray(v_mask, dtype=np.float32),
                    np.asarray(q_mask, dtype=np.float32),
                    np.asarray(Wq, dtype=np.float32),
                    np.asarray(Wk, dtype=np.float32),
                    np.asarray(Wv, dtype=np.float32))
    if profile:
        _install_profile_hook()
    res = run_bass_kernel_spmd(
        nc, maps, list(range(N_CORES)),
        trace=profile, trace_cores=trace_cores,
    )
    out = _assemble(res.results)
    if profile:
        return out, res
    return out


def _install_profile_hook():
    """Wire up the NTFF profile hook that this container image lacks."""
    import types
    if "antenv.axon_hooks" in sys.modules:
        return
    try:
        from trn_agent_boot.trn_boot import _ntff_profile_via_ctypes
        hook = _ntff_profile_via_ctypes("/opt/axon/libaxon_pjrt.so")
    except Exception:
        hook = None
    mod = types.ModuleType("antenv.axon_hooks")
    mod.get_axon_ntff_profile_hook = lambda: hook
    sys.modules["antenv.axon_hooks"] = mod


if __name__ == "__main__":
    t0 = time.time()
    _build()
    print(f"build+compile: {time.time() - t0:.1f}s")


# revision 29
# speedup vs baseline: 1.5722x; 1.0828x over previous
"""Trainium2 Bass kernel for batched multi-head attention (v2, fp16).

Full module:  out = softmax((X_q Wq)(X_k Wk)^T / sqrt(dh) + keymask) (X_v Wv) * qmask
Shapes: B=4, S=2048, D=1024, H=16, dh=64.

Sharding over 8 NeuronCores: core c -> (batch b = c//2, head-group g = c%2).
Each core computes batch b, heads g*8..g*8+8 (Wq/Wk/Wv column-sharded by head).
No collectives; the host scatters inputs (converted to fp16) and gathers the
[2048, 512] fp32 output blocks into the full [4, 2048, 1024] output.

v2 design (vs v1): everything fp16 on the PE.
  - fp16 matmuls run 1 row/cycle with moving-N up to 1024: half the
    instruction count of fp32r (N<=512), and 2x-faster LDWEIGHTS (FWL).
  - Heads are processed in PAIRS: even head's K/Q live on partitions 0:64,
    odd head's on 64:128, so the two S^T matmuls of a pair land on disjoint
    PE row-groups and run concurrently (measured dStart ~4ns on HW).
  - AV is one K=128 matmul per (head, kc) accumulating O^T[65, 1024] in PSUM
    (ones-column appended to VW gives the softmax denominator for free).
  - exp for BOTH heads of a pair is a single F=2048 ACTIVATE (amortizes the
    ~477ns fixed ACT instruction overhead), reading the [128, 2, 1024] S^T
    pair tile from PSUM, writing fp16 P to SBUF.
  - PSUM budget (8 banks): s_pair 4 + o0 2 + o1 2. Tail transposes reuse the
    o slots after their DVE evacuation (same tag, WAR-ordered by Tile).

Per-(pair, q-half) softmax pipeline over 16 k-chunks:
  S^T(kc)   = kw_h^T @ qwT      x2 heads (concurrent row-group pair)
  P(kc)     = exp(S^T * 0.125 + vbias)   one F=2048 ACTIVATE, fp16 out
  O^T      += [VW|1]^T @ P(kc)  x2 heads (K=128, N=1024)
Tail: evacuate O^T (DVE, fp16), PE-transpose [65,128] blocks, normalize
out = O * (qmask/denom) on DVE, DMA out.
"""

import os
import sys
import time
import threading

for _p in ("/opt/trn_rl_repo", "/opt/pypackages"):
    if _p not in sys.path and os.path.isdir(_p):
        sys.path.append(_p)

import numpy as np
from contextlib import ExitStack

import concourse.bass as bass
import concourse.tile as tile
from concourse import bacc, mybir
from concourse.bass_utils import run_bass_kernel_spmd
from concourse.masks import make_identity

B, S, D = 4, 2048, 1024
HEADS, DH = 16, 64
NEG_BIG = 1e10
N_CORES = 8
HG = HEADS // 2          # 8 heads per core
MC = HG * DH             # 512 output cols per core
NSC = S // 128           # 16 seq chunks
NDC = D // 128           # 8 contraction chunks
NMC = MC // 128          # 4 head-dim chunks (of this core's 512 cols)
NKC = NSC                # 16 key chunks
NQH = 2                  # q halves
QH = S // NQH            # 1024

F32 = mybir.dt.float32
EXP = mybir.ActivationFunctionType.Exp

# 16-bit PE dtype: fp16 (default) caps the moving operand at N=512;
# bf16 allows N=1024 (half the matmul instruction count) at ~8x coarser
# element precision. Both run 1 row/cycle on the PE.
USE_BF16 = os.environ.get("KERN_DT", "fp16") == "bf16"
F16 = mybir.dt.bfloat16 if USE_BF16 else mybir.dt.float16
MM_N = 1024 if USE_BF16 else 512
NMM = QH // MM_N         # moving chunks per q-half matmul

if USE_BF16:
    import ml_dtypes
    NP16 = ml_dtypes.bfloat16
else:
    NP16 = np.float16

# AV-side dtype: bf16 allows the P moving operand at N=1024 (one AV matmul
# per head/kc instead of two). P and VW carry ~0.4% element error vs fp16's
# 0.05%, but the S^T scores path stays fp16 so the softmax logits are clean.
BF16 = mybir.dt.bfloat16
I32 = mybir.dt.int32
AV_N = 512
NAV = QH // AV_N

# Schraudolph fast-exp offload: for these k-chunks the softmax exp runs as
#   P = bitcast_f32(i32(round(s * SCHRAU_A + b_k)))   (DVE affine -> Pool copy)
# instead of an ACT ACTIVATE, freeing the saturated ACT engine. ~2% RMS
# relative error on the offloaded P elements (3/16 of them per head; the two
# heads use disjoint chunks so ACT always has the other head's exp to chew).
# Masked keys saturate the i32 convert to -2^31 -> bitcast -0.0 -> correct.
LOG2E = 1.4426950408889634
SCHRAU_A = LOG2E * (1 << 23)
SCHRAU_B = (127.0 - 0.043677) * (1 << 23)
# off by default: DVE+Pool measured slower per element than ACT on HW, and
# the extra engine activity worsened power throttling
if os.environ.get("SCHRAU", "0") == "1":
    SCHRAU_KCS = ({2, 7, 12}, {4, 9, 14})
else:
    SCHRAU_KCS = (set(), set())


def _emit(tc, t):
    nc = tc.nc
    ctx = ExitStack()

    # ---------------- persistent pools ----------------
    cpool = ctx.enter_context(tc.tile_pool(name="const", bufs=1))
    # prefetch the first X_v tiles so the transpose pipeline starts early
    # (V is projected first: attention consumes vw for every k-chunk)
    xv_dram = t["xv"].ap().rearrange("(sc p) d -> sc p d", p=128)
    pre_pool = ctx.enter_context(tc.tile_pool(name="pre", bufs=1))
    pre_x = []
    for i in range(4):
        xpre = pre_pool.tile([128, D], F16, name=f"xpre{i}", tag=f"xpre{i}")
        nc.sync.dma_start(xpre[:], xv_dram[i])
        pre_x.append(xpre)

    ident = cpool.tile([128, 128], F16)
    make_identity(nc, ident[:])
    vbias = cpool.tile([128, NKC], F32)
    nc.sync.dma_start(vbias[:], t["vbias"].ap())
    sbias = cpool.tile([128, NKC], F32)
    nc.sync.dma_start(sbias[:], t["sbias"].ap())
    qmaskT = cpool.tile([128, NSC], F32)
    nc.sync.dma_start(qmaskT[:], t["qmaskT"].ap())

    qk_pool = ctx.enter_context(tc.tile_pool(name="qk", bufs=1))
    qwT = qk_pool.tile([128, NMC, S], F16)        # [m%128, mc, s]
    kwT = qk_pool.tile([128, NMC, S], F16)
    vw = qk_pool.tile([128, NKC, HG, DH + 1], F16)  # [k%128, kc, h, dh|1]
    ones = cpool.tile([128, 1], F32)
    nc.vector.memset(ones[:], 1.0)
    nc.vector.tensor_copy(                           # denominator ones column
        vw[:, :, :, DH:DH + 1], ones[:].broadcast_to([128, NKC, HG, 1])
    )

    # ---------------- projection phase ----------------
    pctx = ExitStack()
    xt_pool = pctx.enter_context(tc.tile_pool(name="xt", bufs=2))
    x_pool = pctx.enter_context(tc.tile_pool(name="x", bufs=12))
    w_pool = pctx.enter_context(tc.tile_pool(name="w", bufs=1))
    psum_t = pctx.enter_context(tc.tile_pool(name="ps_t", bufs=2, space="PSUM"))
    psum_p = pctx.enter_context(tc.tile_pool(name="ps_p", bufs=2, space="PSUM"))

    HSC = NSC // 2  # s-chunks per half

    # stage all weights: QK chunk-contiguous for fast LDWEIGHTS, V moving-major
    w_qk = {}
    for kind in ("q", "k"):
        wt = w_pool.tile([128, NDC, NMC, 128], F16, name=f"w{kind}", tag=f"w{kind}")
        nc.sync.dma_start(
            wt[:],
            t["w" + kind].ap().rearrange("(dc p) (mc m) -> p dc mc m", p=128, m=128),
        )
        w_qk[kind] = wt
    wv_sb = w_pool.tile([128, NDC, MC], F16, tag="wv")
    nc.sync.dma_start(wv_sb[:], t["wv"].ap().rearrange("(dc p) m -> p dc m", p=128))

    def emit_qk_proj(kind, sh, xt, mcs):
        dst = qwT if kind == "q" else kwT
        w_sb = w_qk[kind]
        for mcI in mcs:
            pp = psum_p.tile([128, QH], F32, tag="pp")
            for dc in range(NDC):
                for nh in range(NMM):
                    nc.tensor.matmul(
                        pp[:, nh * MM_N:(nh + 1) * MM_N],
                        w_sb[:, dc, mcI, :],
                        xt[:, dc, nh * MM_N:(nh + 1) * MM_N],
                        start=(dc == 0),
                        stop=(dc == NDC - 1),
                    )
            nc.vector.tensor_copy(dst[:, mcI, sh * QH:(sh + 1) * QH], pp[:])

    for xname, kind in (("xv", "v"), ("xk", "k"), ("xq", "q")):
        x_dram = t[xname].ap().rearrange("(sc p) d -> sc p d", p=128)
        xt_h = []
        for sh in range(2):  # s-halves
            # transpose this half of X into xt [d%128, dc, s_local]
            xt = xt_pool.tile([128, NDC, QH], F16, tag="xt",
                              name=f"xt_{kind}_{sh}")
            xt_h.append(xt)
            for scl in range(HSC):
                sc = sh * HSC + scl
                if xname == "xv" and sh == 0 and scl < len(pre_x):
                    xt_in = pre_x[scl]
                else:
                    xt_in = x_pool.tile([128, D], F16, tag="x")
                    nc.sync.dma_start(xt_in[:], x_dram[sc])
                pt = psum_t.tile([128, NDC, 128], F16, tag="pt")
                for dc in range(NDC):
                    nc.tensor.transpose(
                        pt[:, dc, :], xt_in[:, dc * 128:(dc + 1) * 128], ident[:]
                    )
                nc.vector.tensor_copy(xt[:, :, scl * 128:(scl + 1) * 128], pt[:])

            if kind == "v":
                for scl in range(HSC):
                    sc = sh * HSC + scl
                    pv = psum_p.tile([128, MC], F32, tag="pv")
                    for dc in range(NDC):
                        nc.tensor.matmul(
                            pv[:],
                            xt[:, dc, scl * 128:(scl + 1) * 128],
                            wv_sb[:, dc, :],
                            start=(dc == 0),
                            stop=(dc == NDC - 1),
                        )
                    nc.vector.tensor_copy(
                        vw[:, sc, :, 0:DH],
                        pv[:].rearrange("p (h d) -> p h d", h=HG),
                    )
        if kind in ("q", "k"):
            # both halves of one mc group together, mc0 first: the first
            # attention pair's inputs (mc0) finish before mc1..3 are built
            for mcI in range(NMC):
                for sh in range(2):
                    emit_qk_proj(kind, sh, xt_h[sh], [mcI])

    pctx.close()

    # ---------------- attention phase ----------------
    actx = ExitStack()
    p_pool = actx.enter_context(tc.tile_pool(name="p", bufs=3))
    y_pool = actx.enter_context(tc.tile_pool(name="y", bufs=2))
    ot_pool = actx.enter_context(tc.tile_pool(name="ot", bufs=4))
    rq_pool = actx.enter_context(tc.tile_pool(name="rq", bufs=2))
    out_pool = actx.enter_context(tc.tile_pool(name="out", bufs=3))
    psum_s = actx.enter_context(tc.tile_pool(name="ps_s", bufs=2, space="PSUM"))
    psum_o = actx.enter_context(tc.tile_pool(name="ps_o", bufs=2, space="PSUM"))

    # DRAM view: [qh, p, qb, h, d] for per-(head, q-half) strip stores
    out_v = t["out"].ap().rearrange(
        "(a qb p) (hh d) -> a p qb hh d", a=NQH, p=128, hh=HG
    )

    # deferred tail work (PE transposes + normalize) from the previous
    # (pair, qh) iteration; flushed early in the next iteration so the
    # transposes fill the PE gap while ACT streams the first exps.
    tails = []

    def flush_tail():
        while tails:
            tails.pop(0)()

    for hp in range(HG // 2):
        h0, h1 = 2 * hp, 2 * hp + 1
        mcI = hp                      # mc_h = h // 2 == hp for both heads
        kw0 = kwT[0:64, mcI, :]
        kw1 = kwT[64:128, mcI, :]
        qw0 = qwT[0:64, mcI, :]
        qw1 = qwT[64:128, mcI, :]
        for qh in range(NQH):
            q0 = qh * QH
            # per-head S^T tiles (2 banks each) and P tiles; the two heads'
            # exps are STAGGERED on ACT so each head's S(kc+1)/AV(kc) runs on
            # the PE under the OTHER head's exp -> ACT never idles.
            s_t = [
                psum_s.tile([128, QH], F32, tag="s", name=f"s{i}_{hp}_{qh}")
                for i in range(2)
            ]
            kwh = (kw0, kw1)
            qwh = (qw0, qw1)

            def emit_S(i, kc):
                for nh in range(NMM):
                    nc.tensor.matmul(
                        s_t[i][:, nh * MM_N:(nh + 1) * MM_N],
                        kwh[i][:, kc * 128:(kc + 1) * 128],
                        qwh[i][:, q0 + nh * MM_N:q0 + (nh + 1) * MM_N],
                        start=True, stop=True,
                    )

            def emit_exp(i, kc):
                p_t = p_pool.tile([128, QH], F16, tag="p",
                                  name=f"p{i}_{hp}_{qh}_{kc}")
                if kc in SCHRAU_KCS[i]:
                    yi = y_pool.tile([128, QH], I32, tag="y",
                                     name=f"y{i}_{hp}_{qh}_{kc}")
                    nc.vector.tensor_scalar(
                        yi[:], s_t[i][:], SCHRAU_A * 0.125,
                        sbias[:, kc:kc + 1],
                        mybir.AluOpType.mult, mybir.AluOpType.add,
                    )
                    nc.gpsimd.tensor_copy(p_t[:], yi[:].bitcast(F32))
                else:
                    nc.scalar.activation(
                        p_t[:], s_t[i][:], EXP,
                        bias=vbias[:, kc:kc + 1], scale=0.125,
                    )
                return p_t

            emit_S(0, 0)
            emit_S(1, 0)
            p0 = emit_exp(0, 0)
            # previous iteration's transposes/normalize fill the PE bubble
            # while ACT runs this iteration's first exps
            flush_tail()
            o_t = [
                psum_o.tile([DH + 1, QH], F32, tag="o", name=f"o{i}_{hp}_{qh}")
                for i in range(2)
            ]

            def emit_av(i, kc, p_t):
                first, last = kc == 0, kc == NKC - 1
                for nh in range(NAV):
                    nc.tensor.matmul(
                        o_t[i][:, nh * AV_N:(nh + 1) * AV_N],
                        vw[:, kc, 2 * hp + i, :],
                        p_t[:, nh * AV_N:(nh + 1) * AV_N],
                        start=first, stop=last,
                    )

            for kc in range(NKC):
                if kc > 0:
                    p0 = emit_exp(0, kc)
                if kc + 1 < NKC:
                    emit_S(0, kc + 1)
                emit_av(0, kc, p0)
                p1 = emit_exp(1, kc)
                if kc + 1 < NKC:
                    emit_S(1, kc + 1)
                emit_av(1, kc, p1)

            # evacuate O^T now (frees the o slots for the next iteration);
            # defer transposes + normalize into the next iteration's start.
            evac = []
            for i in range(2):
                ot = ot_pool.tile([DH + 1, QH], F16, tag="ot",
                                  name=f"ot_{2 * hp + i}_{qh}")
                nc.vector.tensor_copy(ot[:], o_t[i][:])
                evac.append(ot)

            def make_tail(hp=hp, qh=qh, evac=evac):
                def tail():
                    for hloc, ot in ((2 * hp, evac[0]), (2 * hp + 1, evac[1])):
                        # inner dim padded to DH+2 so each [*, qb, :] slice
                        # lands 4-byte aligned in PSUM (fp16 elements)
                        tr = psum_o.tile([128, 8, DH + 2], F16, tag="o",
                                         name=f"tr_{hloc}_{qh}")
                        for qb in range(8):
                            nc.tensor.transpose(
                                tr[:, qb, 0:DH + 1],
                                ot[:, qb * 128:(qb + 1) * 128],
                                ident[0:DH + 1, 0:DH + 1],
                            )
                        # normalize: out = O * qmask/denom (denom = col 64)
                        rq = rq_pool.tile([128, 8], F32, tag="rq",
                                          name=f"rq_{hloc}_{qh}")
                        nc.vector.reciprocal(rq[:], tr[:, :, DH])
                        nc.vector.tensor_mul(
                            rq[:], rq[:], qmaskT[:, qh * 8:(qh + 1) * 8]
                        )
                        ob = out_pool.tile([128, 8, DH], F32, tag="ob",
                                           name=f"ob_{hloc}_{qh}")
                        nc.vector.tensor_mul(
                            ob[:], tr[:, :, 0:DH],
                            rq[:].broadcast_to([128, 8, DH]),
                        )
                        nc.sync.dma_start(out_v[qh][:, :, hloc, :], ob[:])
                return tail

            tails.append(make_tail())

    flush_tail()
    actx.close()
    ctx.close()


_BUILD_LOCK = threading.Lock()
_CACHE = {}


def _build():
    with _BUILD_LOCK:
        if "nc" in _CACHE:
            return _CACHE["nc"]
        nc = bacc.Bacc(
            "TRN2", target_bir_lowering=False, debug=False, num_devices=N_CORES
        )
        t = {
            "xq": nc.dram_tensor("xq", [S, D], F16, kind="ExternalInput"),
            "xk": nc.dram_tensor("xk", [S, D], F16, kind="ExternalInput"),
            "xv": nc.dram_tensor("xv", [S, D], F16, kind="ExternalInput"),
            "wq": nc.dram_tensor("wq", [D, MC], F16, kind="ExternalInput"),
            "wk": nc.dram_tensor("wk", [D, MC], F16, kind="ExternalInput"),
            "wv": nc.dram_tensor("wv", [D, MC], F16, kind="ExternalInput"),
            "vbias": nc.dram_tensor("vbias", [128, NKC], F32, kind="ExternalInput"),
            "sbias": nc.dram_tensor("sbias", [128, NKC], F32, kind="ExternalInput"),
            "qmaskT": nc.dram_tensor("qmaskT", [128, NSC], F32, kind="ExternalInput"),
            "out": nc.dram_tensor("out", [S, MC], F32, kind="ExternalOutput"),
        }
        with tile.TileContext(nc) as tc:
            _emit(tc, t)
        nc.compile()
        _CACHE["nc"] = nc
        return nc


def _in_maps(q_value, k_value, v_value, v_mask, q_mask, Wq, Wk, Wv):
    maps = []
    for c in range(N_CORES):
        b, g = c // 2, c % 2
        m0 = g * MC
        vb = ((v_mask[b, :, 0].reshape(NKC, 128).T) - 1.0) * NEG_BIG
        qm = q_mask[b, :, 0].reshape(NSC, 128).T
        maps.append({
            "xq": np.ascontiguousarray(q_value[b]).astype(NP16),
            "xk": np.ascontiguousarray(k_value[b]).astype(NP16),
            "xv": np.ascontiguousarray(v_value[b]).astype(NP16),
            "wq": np.ascontiguousarray(Wq[:, m0:m0 + MC]).astype(NP16),
            "wk": np.ascontiguousarray(Wk[:, m0:m0 + MC]).astype(NP16),
            "wv": np.ascontiguousarray(Wv[:, m0:m0 + MC]).astype(NP16),
            "vbias": np.ascontiguousarray(vb).astype(np.float32),
            "sbias": np.ascontiguousarray(
                vb * 0.125 * SCHRAU_A + SCHRAU_B).astype(np.float32),
            "qmaskT": np.ascontiguousarray(qm).astype(np.float32),
        })
    return maps


def _assemble(results):
    out = np.empty((B, S, HEADS * DH), dtype=np.float32)
    for c in range(N_CORES):
        b, g = c // 2, c % 2
        out[b, :, g * MC:(g + 1) * MC] = results[c]["out"]
    return out


def kernel(q_value, k_value, v_value, v_mask, q_mask, Wq, Wk, Wv,
           profile=False, trace_cores=None):
    nc = _build()
    maps = _in_maps(np.asarray(q_value, dtype=np.float32),
                    np.asarray(k_value, dtype=np.float32),
                    np.asarray(v_value, dtype=np.float32),
                    np.asarray(v_mask, dtype=np.float32),
                    np.asarray(q_mask, dtype=np.float32),
                    np.asarray(Wq, dtype=np.float32),
                    np.asarray(Wk, dtype=np.float32),
                    np.asarray(Wv, dtype=np.float32))
    if profile:
        _install_profile_hook()
    res = run_bass_kernel_spmd(
        nc, maps, list(range(N_CORES)),
        trace=profile, trace_cores=trace_cores,
    )
    out = _assemble(res.results)
    if profile:
        return out, res
    return out


def _install_profile_hook():
    """Wire up the NTFF profile hook that this container image lacks."""
    import types
    if "antenv.axon_hooks" in sys.modules:
        return
    try:
        from trn_agent_boot.trn_boot import _ntff_profile_via_ctypes
        hook = _ntff_profile_via_ctypes("/opt/axon/libaxon_pjrt.so")
    except Exception:
        hook = None
    mod = types.ModuleType("antenv.axon_hooks")
    mod.get_axon_ntff_profile_hook = lambda: hook
    sys.modules["antenv.axon_hooks"] = mod


if __name__ == "__main__":
    t0 = time.time()
    _build()
    print(f"build+compile: {time.time() - t0:.1f}s")
